# revision 19
# baseline (speedup 1.0000x reference)
"""CapsuleMaxPooling Trainium2 kernel.

Problem: inp [B=32, C=32, H=64, W=64, D=8] f32, kernel_size k=2.
For each 2x2 spatial window pick the capsule vector (length D=8) with the
largest squared L2 norm (first-max tie-break) -> out [B, C, 32, 32, 8].

Strategy (fully data-parallel, shard B across 8 cores; per core the shard is
viewed as rows r=(b, c, hk) of 1024 contiguous floats = (dh, wk, dw, d).
Rows are assigned to partitions block-contiguously (partition p owns rows
r0+p*tb..r0+p*tb+tb-1 of a batch) so each partition's DMA side is one large
contiguous descriptor.

Engine split:
  - ACT: sq = x^2 (Square activation, output pre-rounded to float32r) and
    the base copy of candidate D into the output tile.
  - PE: the d=8 norm reduction as 8 PSUM-accumulated identity matmuls per
    batch (float32r single-pass mode; N = tb*128 <= 512, bank-aligned
    slices of a per-group PSUM tile). ~19-bit norms are plenty to pick the
    right window (the rel-err gate is 2e-2; measured ~1e-2).
  - DVE: the mask tournament reading norms straight from PSUM (no
    evacuation pass), then per batch a merged C/B copy_predicated (single
    op via a negative-stride candidate axis; B processed after C keeps
    first-max order) and the A copy_predicated.
  - copy_predicated wants an integer mask: int32 bitcast view of the f32
    0.0/1.0 mask (1.0f = 0x3F800000 != 0) broadcast over d via stride-0.

The per-group work is emitted software-pipelined: stage A (input DMA,
square, matmuls, base copy) of group g+1 is emitted before stage C
(tournament, copy_predicated, output DMA) of group g, so no engine queue
ever sits blocked on a cross-engine semaphore while it still has
independent work (in-order queues would otherwise stall, which serialized
the whole tail of the previous version).
"""

import numpy as np

try:
    import concourse.bass as bass
except ImportError:  # pragma: no cover
    import sys

    sys.path.insert(0, "/opt/trn_rl_repo")
    import concourse.bass as bass

from concourse import bacc, mybir
from concourse.bass_utils import run_bass_kernel_spmd
from concourse.masks import make_identity
from concourse.tile import TileContext

P = 128
N_CORES = 8
ROW_W = 1024  # (dh=2) * (wk=32) * (dw=2) * (d=8)
OUT_W = 256  # (wk=32) * (d=8)
# row-tiles per batch; sums to R // P (= 32). Small batches at both edges
# shorten pipeline ramp-in and ramp-out.
DEFAULT_SCHED = (1, 1, 2, 2, 4, 4, 4, 4, 4, 2, 2, 1, 1)


def _ap(t, extra_off, dims):
    """Build an AP on tile t's underlying tensor: partition dim from t,
    then explicit [step, count] dims."""
    return bass.AP(tensor=t.tensor, offset=t.offset + extra_off, ap=[t.ap[0], *dims])


def build_nc(R=4096, sched=DEFAULT_SCHED, GM=2):
    """Build the per-core Bass program. R = rows (b,c,hk) per core."""
    f32 = mybir.dt.float32
    i32 = mybir.dt.int32
    f32r = mybir.dt.float32r
    mx = mybir.AluOpType.max
    ge = mybir.AluOpType.is_ge
    nc = bacc.Bacc(None, target_bir_lowering=False)
    x = nc.dram_tensor("x", [R, ROW_W], f32, kind="ExternalInput")
    y = nc.dram_tensor("y", [R, OUT_W], f32, kind="ExternalOutput")
    assert sum(sched) * P == R
    groups = [list(sched[i : i + GM]) for i in range(0, len(sched), GM)]

    with TileContext(nc) as tc:
        with (
            tc.tile_pool(name="constp", bufs=1) as constp,
            tc.tile_pool(name="xp", bufs=7) as xp,
            tc.tile_pool(name="sqp", bufs=2) as sqp,
            tc.tile_pool(name="maskp", bufs=2) as maskp,
            tc.tile_pool(name="outp", bufs=6) as outp,
            tc.psum_pool(name="npp", bufs=3) as npp,
        ):
            ident0 = constp.tile([P, P], f32, tag="ident0")
            make_identity(nc, ident0)
            ident = constp.tile([P, P], f32r, tag="ident")
            nc.scalar.copy(ident, ident0)

            def stage_a(grp, tile0):
                """Input DMA, square, norm matmuls, base copy for one group.
                Returns state for stage_c."""
                gtb = sum(grp)
                np_t = npp.tile([P, gtb, 128], f32, tag="np")
                xts, ots, qoff = [], [], [0]
                for tb in grp:
                    r0 = tile0 * P
                    xt = xp.tile([P, tb, ROW_W], f32, tag="xt")
                    xts.append(xt)
                    nc.sync.dma_start(
                        out=xt,
                        in_=x[r0 : r0 + tb * P, :].rearrange(
                            "(p j) c -> p j c", j=tb
                        ),
                    )
                    sq = sqp.tile([P, tb, ROW_W], f32r, tag="sq")
                    nc.scalar.square(sq, xt)
                    q0 = qoff[-1]
                    sqv = sq.rearrange(
                        "p j (dh wk dw dd) -> p j dh wk dw dd", dh=2, dw=2, dd=8
                    )
                    npv = np_t[:, q0 : q0 + tb].rearrange(
                        "p j (dh wk dw) -> p j dh wk dw", dh=2, dw=2
                    )
                    for di in range(8):
                        nc.tensor.matmul(
                            npv,
                            ident,
                            sqv[:, :, :, :, :, di],
                            start=(di == 0),
                            stop=(di == 7),
                        )
                    ot = outp.tile([P, tb, 32, 8], f32, tag="ot")
                    ots.append(ot)
                    xr = xt.rearrange(
                        "p j (dh wk dw d) -> p j dh wk dw d", dh=2, dw=2, d=8
                    )
                    # base copy of candidate D on the otherwise-idle GpSimd
                    nc.gpsimd.tensor_copy(ot, xr[:, :, 1, :, 1, :])
                    qoff.append(q0 + tb)
                    tile0 += tb
                return (grp, tile0 - gtb, np_t, xts, ots, qoff)

            def stage_c(state):
                """Tournament, predicated selection, output DMA for one
                group (runs one group behind stage_a)."""
                grp, t0, np_t, xts, ots, qoff = state
                gtb = sum(grp)
                # norms layout per row: (dh, wk, dw); candidate offsets
                # A=(0,w,0)->2w, B=(0,w,1)->1+2w, C=(1,w,0)->64+2w,
                # D=(1,w,1)->65+2w
                nst = np_t.ap[1][0]  # per-j stride (=128)
                nA = _ap(np_t, 0, [[nst, gtb], [2, 32]])

                # only ONE TensorTensor input may come from PSUM, so the
                # first tournament level is a (single-input) max-reduce over
                # the dw pairs: h12[:, :, dh, wk] = max over dw.
                h12 = maskp.tile([P, gtb, 2, 32], f32, tag="h12")
                nc.vector.tensor_reduce(
                    h12.rearrange("p j dh wk -> p (j dh) wk"),
                    np_t.rearrange(
                        "p j (dh wk dw) -> p (j dh) wk dw", dh=2, dw=2
                    ),
                    axis=mybir.AxisListType.X,
                    op=mx,
                )
                M = maskp.tile([P, gtb, 32], f32, tag="M")
                nc.vector.tensor_tensor(
                    M, h12[:, :, 0], h12[:, :, 1], op=mx
                )
                wA = maskp.tile([P, gtb, 32], f32, tag="wA")
                nc.vector.tensor_tensor(wA, nA, M, op=ge)
                # combined C/B mask [P, gtb, 2, 32]: cand axis steps C -> B
                # (norm offset 64 -> 1, step -63)
                wCB = maskp.tile([P, gtb, 2, 32], f32, tag="wCB")
                nCB = _ap(np_t, 64, [[nst, gtb], [-63, 2], [2, 32]])
                Mb = _ap(M, 0, [[M.ap[1][0], gtb], [0, 2], [1, 32]])
                nc.vector.tensor_tensor(wCB, nCB, Mb, op=ge)

                tile1 = t0
                for qi, tb in enumerate(grp):
                    r0 = tile1 * P
                    xt = xts[qi]
                    ot = ots[qi]
                    q0 = qoff[qi]
                    jst = xt.ap[1][0]  # per-j stride (=1024)
                    ost = ot.ap[1][0]  # per-j stride (=256)
                    oV = _ap(ot, 0, [[ost, tb], [8, 32], [1, 8]])

                    def mask(w, off_elems, step):
                        a = w[:, q0 : q0 + tb].bitcast(i32)
                        return bass.AP(
                            tensor=a.tensor,
                            offset=a.offset + off_elems,
                            ap=[a.ap[0], [a.ap[1][0], tb], [step, 32], [0, 8]],
                        )

                    # copy_predicated is limited to 3 free dims, so C, B, A
                    # run as separate ops (overwrite order gives first-max).
                    dC = _ap(xt, 512, [[jst, tb], [16, 32], [1, 8]])
                    nc.vector.copy_predicated(oV, mask(wCB, 0, 1), dC)
                    dB = _ap(xt, 8, [[jst, tb], [16, 32], [1, 8]])
                    nc.vector.copy_predicated(oV, mask(wCB, 32, 1), dB)
                    dA = _ap(xt, 0, [[jst, tb], [16, 32], [1, 8]])
                    nc.vector.copy_predicated(oV, mask(wA, 0, 1), dA)

                    nc.sync.dma_start(
                        out=y[r0 : r0 + tb * P, :].rearrange(
                            "(p j) c -> p j c", j=tb
                        ),
                        in_=ot.rearrange("p j w d -> p j (w d)"),
                    )
                    tile1 += tb

            pending = None
            tile0 = 0
            for grp in groups:
                state = stage_a(grp, tile0)
                tile0 += sum(grp)
                if pending is not None:
                    stage_c(pending)
                pending = state
            stage_c(pending)
    nc.compile()
    return nc


_NC_CACHE = {}


def _get_nc(R):
    if R not in _NC_CACHE:
        _NC_CACHE[R] = build_nc(R)
    return _NC_CACHE[R]


def kernel(inp, kernel_size):
    inp = np.asarray(inp)
    k = int(np.asarray(kernel_size))
    assert k == 2, f"kernel hardcoded for kernel_size=2, got {k}"
    B, C, H, W, D = inp.shape
    assert (B, C, H, W, D) == (32, 32, 64, 64, 8), inp.shape
    Hk, Wk = H // k, W // k

    bs = B // N_CORES  # 4 batches per core
    R = bs * C * Hk  # 4096 rows per core
    nc = _get_nc(R)

    in_maps = []
    for c in range(N_CORES):
        shard = np.ascontiguousarray(inp[c * bs : (c + 1) * bs]).reshape(R, ROW_W)
        in_maps.append({"x": shard})

    res = run_bass_kernel_spmd(nc, in_maps, list(range(N_CORES)))
    out = np.concatenate(
        [r["y"].reshape(bs, C, Hk, Wk, D) for r in res.results], axis=0
    )
    return out


# revision 20
# speedup vs baseline: 1.3051x; 1.3051x over previous
"""CapsuleMaxPooling Trainium2 kernel.

Problem: inp [B=32, C=32, H=64, W=64, D=8] f32, kernel_size k=2.
For each 2x2 spatial window pick the capsule vector (length D=8) with the
largest squared L2 norm (first-max tie-break) -> out [B, C, 32, 32, 8].

Strategy (fully data-parallel, shard B across 8 cores; per core the shard is
viewed as rows r=(b, c, hk) of 1024 contiguous floats = (dh, wk, dw, d).
Rows are assigned to partitions block-contiguously (partition p owns rows
r0+p*tb..r0+p*tb+tb-1 of a batch) so each partition's DMA side is one large
contiguous descriptor.

Engine split:
  - ACT: sq = x^2 (Square activation, output pre-rounded to float32r) and
    the base copy of candidate D into the output tile.
  - PE: the d=8 norm reduction as 8 PSUM-accumulated identity matmuls per
    batch (float32r single-pass mode; N = tb*128 <= 512, bank-aligned
    slices of a per-group PSUM tile). ~19-bit norms are plenty to pick the
    right window (the rel-err gate is 2e-2; measured ~1e-2).
  - DVE: the mask tournament reading norms straight from PSUM (no
    evacuation pass), then per batch a merged C/B copy_predicated (single
    op via a negative-stride candidate axis; B processed after C keeps
    first-max order) and the A copy_predicated.
  - copy_predicated wants an integer mask: int32 bitcast view of the f32
    0.0/1.0 mask (1.0f = 0x3F800000 != 0) broadcast over d via stride-0.

The per-group work is emitted software-pipelined: stage A (input DMA,
square, matmuls, base copy) of group g+1 is emitted before stage C
(tournament, copy_predicated, output DMA) of group g, so no engine queue
ever sits blocked on a cross-engine semaphore while it still has
independent work (in-order queues would otherwise stall, which serialized
the whole tail of the previous version).
"""

import numpy as np

try:
    import concourse.bass as bass
except ImportError:  # pragma: no cover
    import sys

    sys.path.insert(0, "/opt/trn_rl_repo")
    import concourse.bass as bass

from concourse import bacc, mybir
from concourse.bass_utils import run_bass_kernel_spmd
from concourse.masks import make_identity
from concourse.tile import TileContext

P = 128
N_CORES = 8
ROW_W = 1024  # (dh=2) * (wk=32) * (dw=2) * (d=8)
OUT_W = 256  # (wk=32) * (d=8)
# row-tiles per batch; sums to R // P (= 32). Small batches at both edges
# shorten pipeline ramp-in and ramp-out.
DEFAULT_SCHED = (1, 1, 2, 2, 4, 4, 4, 4, 4, 2, 2, 1, 1)


def _ap(t, extra_off, dims):
    """Build an AP on tile t's underlying tensor: partition dim from t,
    then explicit [step, count] dims."""
    return bass.AP(tensor=t.tensor, offset=t.offset + extra_off, ap=[t.ap[0], *dims])


def build_nc(R=4096, sched=DEFAULT_SCHED, GM=2):
    """Build the per-core Bass program. R = rows (b,c,hk) per core."""
    f32 = mybir.dt.float32
    i32 = mybir.dt.int32
    f32r = mybir.dt.float32r
    mx = mybir.AluOpType.max
    ge = mybir.AluOpType.is_ge
    nc = bacc.Bacc(None, target_bir_lowering=False)
    x = nc.dram_tensor("x", [R, ROW_W], f32, kind="ExternalInput")
    y = nc.dram_tensor("y", [R, OUT_W], f32, kind="ExternalOutput")
    assert sum(sched) * P == R
    groups = [list(sched[i : i + GM]) for i in range(0, len(sched), GM)]

    with TileContext(nc) as tc:
        with (
            tc.tile_pool(name="constp", bufs=1) as constp,
            tc.tile_pool(name="xp", bufs=7) as xp,
            tc.tile_pool(name="sqp", bufs=2) as sqp,
            tc.tile_pool(name="maskp", bufs=2) as maskp,
            tc.tile_pool(name="outp", bufs=6) as outp,
            tc.psum_pool(name="npp", bufs=3) as npp,
        ):
            ident0 = constp.tile([P, P], f32, tag="ident0")
            make_identity(nc, ident0)
            ident = constp.tile([P, P], f32r, tag="ident")
            nc.scalar.copy(ident, ident0)

            def stage_a(grp, tile0):
                """Input DMA, square, norm matmuls, base copy for one group.
                Returns state for stage_c."""
                gtb = sum(grp)
                np_t = npp.tile([P, gtb, 128], f32, tag="np")
                xts, ots, qoff = [], [], [0]
                for tb in grp:
                    r0 = tile0 * P
                    xt = xp.tile([P, tb, ROW_W], f32, tag="xt")
                    xts.append(xt)
                    nc.sync.dma_start(
                        out=xt,
                        in_=x[r0 : r0 + tb * P, :].rearrange(
                            "(p j) c -> p j c", j=tb
                        ),
                    )
                    sq = sqp.tile([P, tb, ROW_W], f32r, tag="sq")
                    nc.scalar.square(sq, xt)
                    q0 = qoff[-1]
                    sqv = sq.rearrange(
                        "p j (dh wk dw dd) -> p j dh wk dw dd", dh=2, dw=2, dd=8
                    )
                    npv = np_t[:, q0 : q0 + tb].rearrange(
                        "p j (dh wk dw) -> p j dh wk dw", dh=2, dw=2
                    )
                    for di in range(8):
                        nc.tensor.matmul(
                            npv,
                            ident,
                            sqv[:, :, :, :, :, di],
                            start=(di == 0),
                            stop=(di == 7),
                        )
                    ot = outp.tile([P, tb, 32, 8], f32, tag="ot")
                    ots.append(ot)
                    xr = xt.rearrange(
                        "p j (dh wk dw d) -> p j dh wk dw d", dh=2, dw=2, d=8
                    )
                    nc.scalar.copy(ot, xr[:, :, 1, :, 1, :])
                    qoff.append(q0 + tb)
                    tile0 += tb
                return (grp, tile0 - gtb, np_t, xts, ots, qoff)

            def stage_c(state):
                """Tournament, predicated selection, output DMA for one
                group (runs one group behind stage_a)."""
                grp, t0, np_t, xts, ots, qoff = state
                gtb = sum(grp)
                # norms layout per row: (dh, wk, dw); candidate offsets
                # A=(0,w,0)->2w, B=(0,w,1)->1+2w, C=(1,w,0)->64+2w,
                # D=(1,w,1)->65+2w
                nst = np_t.ap[1][0]  # per-j stride (=128)
                nA = _ap(np_t, 0, [[nst, gtb], [2, 32]])

                # only ONE TensorTensor input may come from PSUM, so the
                # first tournament level is a (single-input) max-reduce over
                # the dw pairs: h12[:, :, dh, wk] = max over dw.
                h12 = maskp.tile([P, gtb, 2, 32], f32, tag="h12")
                nc.vector.tensor_reduce(
                    h12.rearrange("p j dh wk -> p (j dh) wk"),
                    np_t.rearrange(
                        "p j (dh wk dw) -> p (j dh) wk dw", dh=2, dw=2
                    ),
                    axis=mybir.AxisListType.X,
                    op=mx,
                )
                M = maskp.tile([P, gtb, 32], f32, tag="M")
                nc.vector.tensor_tensor(
                    M, h12[:, :, 0], h12[:, :, 1], op=mx
                )
                wA = maskp.tile([P, gtb, 32], f32, tag="wA")
                nc.vector.tensor_tensor(wA, nA, M, op=ge)
                # combined C/B mask [P, gtb, 2, 32]: cand axis steps C -> B
                # (norm offset 64 -> 1, step -63)
                wCB = maskp.tile([P, gtb, 2, 32], f32, tag="wCB")
                nCB = _ap(np_t, 64, [[nst, gtb], [-63, 2], [2, 32]])
                Mb = _ap(M, 0, [[M.ap[1][0], gtb], [0, 2], [1, 32]])
                nc.vector.tensor_tensor(wCB, nCB, Mb, op=ge)

                tile1 = t0
                for qi, tb in enumerate(grp):
                    r0 = tile1 * P
                    xt = xts[qi]
                    ot = ots[qi]
                    q0 = qoff[qi]
                    jst = xt.ap[1][0]  # per-j stride (=1024)
                    ost = ot.ap[1][0]  # per-j stride (=256)
                    oV = _ap(ot, 0, [[ost, tb], [8, 32], [1, 8]])

                    def mask(w, off_elems, step):
                        a = w[:, q0 : q0 + tb].bitcast(i32)
                        return bass.AP(
                            tensor=a.tensor,
                            offset=a.offset + off_elems,
                            ap=[a.ap[0], [a.ap[1][0], tb], [step, 32], [0, 8]],
                        )

                    # copy_predicated is limited to 3 free dims, so C, B, A
                    # run as separate ops (overwrite order gives first-max).
                    dC = _ap(xt, 512, [[jst, tb], [16, 32], [1, 8]])
                    nc.vector.copy_predicated(oV, mask(wCB, 0, 1), dC)
                    dB = _ap(xt, 8, [[jst, tb], [16, 32], [1, 8]])
                    nc.vector.copy_predicated(oV, mask(wCB, 32, 1), dB)
                    dA = _ap(xt, 0, [[jst, tb], [16, 32], [1, 8]])
                    nc.vector.copy_predicated(oV, mask(wA, 0, 1), dA)

                    nc.sync.dma_start(
                        out=y[r0 : r0 + tb * P, :].rearrange(
                            "(p j) c -> p j c", j=tb
                        ),
                        in_=ot.rearrange("p j w d -> p j (w d)"),
                    )
                    tile1 += tb

            pending = None
            tile0 = 0
            for grp in groups:
                state = stage_a(grp, tile0)
                tile0 += sum(grp)
                if pending is not None:
                    stage_c(pending)
                pending = state
            stage_c(pending)
    nc.compile()
    return nc


_NC_CACHE = {}


def _get_nc(R):
    if R not in _NC_CACHE:
        _NC_CACHE[R] = build_nc(R)
    return _NC_CACHE[R]


def kernel(inp, kernel_size):
    inp = np.asarray(inp)
    k = int(np.asarray(kernel_size))
    assert k == 2, f"kernel hardcoded for kernel_size=2, got {k}"
    B, C, H, W, D = inp.shape
    assert (B, C, H, W, D) == (32, 32, 64, 64, 8), inp.shape
    Hk, Wk = H // k, W // k

    bs = B // N_CORES  # 4 batches per core
    R = bs * C * Hk  # 4096 rows per core
    nc = _get_nc(R)

    in_maps = []
    for c in range(N_CORES):
        shard = np.ascontiguousarray(inp[c * bs : (c + 1) * bs]).reshape(R, ROW_W)
        in_maps.append({"x": shard})

    res = run_bass_kernel_spmd(nc, in_maps, list(range(N_CORES)))
    out = np.concatenate(
        [r["y"].reshape(bs, C, Hk, Wk, D) for r in res.results], axis=0
    )
    return out


# revision 21
# speedup vs baseline: 1.3417x; 1.0280x over previous
"""CapsuleMaxPooling Trainium2 kernel.

Problem: inp [B=32, C=32, H=64, W=64, D=8] f32, kernel_size k=2.
For each 2x2 spatial window pick the capsule vector (length D=8) with the
largest squared L2 norm (first-max tie-break) -> out [B, C, 32, 32, 8].

Strategy (fully data-parallel, shard B across 8 cores; per core the shard is
viewed as rows r=(b, c, hk) of 1024 contiguous floats = (dh, wk, dw, d).
Rows are assigned to partitions block-contiguously (partition p owns rows
r0+p*tb..r0+p*tb+tb-1 of a batch) so each partition's DMA side is one large
contiguous descriptor.

Engine split:
  - ACT: sq = x^2 (Square activation, output pre-rounded to float32r) and
    the base copy of candidate D into the output tile.
  - PE: the d=8 norm reduction as 8 PSUM-accumulated identity matmuls per
    batch (float32r single-pass mode; N = tb*128 <= 512, bank-aligned
    slices of a per-group PSUM tile). ~19-bit norms are plenty to pick the
    right window (the rel-err gate is 2e-2; measured ~1e-2).
  - DVE: the mask tournament reading norms straight from PSUM (no
    evacuation pass), then per batch a merged C/B copy_predicated (single
    op via a negative-stride candidate axis; B processed after C keeps
    first-max order) and the A copy_predicated.
  - copy_predicated wants an integer mask: int32 bitcast view of the f32
    0.0/1.0 mask (1.0f = 0x3F800000 != 0) broadcast over d via stride-0.

The per-group work is emitted software-pipelined: stage A (input DMA,
square, matmuls, base copy) of group g+1 is emitted before stage C
(tournament, copy_predicated, output DMA) of group g, so no engine queue
ever sits blocked on a cross-engine semaphore while it still has
independent work (in-order queues would otherwise stall, which serialized
the whole tail of the previous version).
"""

import numpy as np

try:
    import concourse.bass as bass
except ImportError:  # pragma: no cover
    import sys

    sys.path.insert(0, "/opt/trn_rl_repo")
    import concourse.bass as bass

from concourse import bacc, mybir
from concourse.bass_utils import run_bass_kernel_spmd
from concourse.masks import make_identity
from concourse.tile import TileContext

P = 128
N_CORES = 8
ROW_W = 1024  # (dh=2) * (wk=32) * (dw=2) * (d=8)
OUT_W = 256  # (wk=32) * (d=8)
# row-tiles per batch; sums to R // P (= 32). Small batches at both edges
# shorten pipeline ramp-in and ramp-out.
DEFAULT_SCHED = (1, 1, 2, 2, 4, 4, 4, 4, 4, 2, 2, 1, 1)


def _ap(t, extra_off, dims):
    """Build an AP on tile t's underlying tensor: partition dim from t,
    then explicit [step, count] dims."""
    return bass.AP(tensor=t.tensor, offset=t.offset + extra_off, ap=[t.ap[0], *dims])


def build_nc(R=4096, sched=DEFAULT_SCHED, GM=2):
    """Build the per-core Bass program. R = rows (b,c,hk) per core."""
    f32 = mybir.dt.float32
    i32 = mybir.dt.int32
    f32r = mybir.dt.float32r
    mx = mybir.AluOpType.max
    ge = mybir.AluOpType.is_ge
    nc = bacc.Bacc(None, target_bir_lowering=False)
    x = nc.dram_tensor("x", [R, ROW_W], f32, kind="ExternalInput")
    y = nc.dram_tensor("y", [R, OUT_W], f32, kind="ExternalOutput")
    assert sum(sched) * P == R
    groups = [list(sched[i : i + GM]) for i in range(0, len(sched), GM)]

    with TileContext(nc) as tc:
        with (
            tc.tile_pool(name="constp", bufs=1) as constp,
            tc.tile_pool(name="xp", bufs=7) as xp,
            tc.tile_pool(name="sqp", bufs=3) as sqp,
            tc.tile_pool(name="maskp", bufs=2) as maskp,
            tc.tile_pool(name="outp", bufs=6) as outp,
            tc.psum_pool(name="npp", bufs=3) as npp,
        ):
            ident0 = constp.tile([P, P], f32, tag="ident0")
            make_identity(nc, ident0)
            ident = constp.tile([P, P], f32r, tag="ident")
            nc.scalar.copy(ident, ident0)

            def stage_a(grp, tile0):
                """Input DMA, square, norm matmuls, base copy for one group.
                Returns state for stage_c."""
                gtb = sum(grp)
                np_t = npp.tile([P, gtb, 128], f32, tag="np")
                xts, ots, qoff = [], [], [0]
                for tb in grp:
                    r0 = tile0 * P
                    xt = xp.tile([P, tb, ROW_W], f32, tag="xt")
                    xts.append(xt)
                    nc.sync.dma_start(
                        out=xt,
                        in_=x[r0 : r0 + tb * P, :].rearrange(
                            "(p j) c -> p j c", j=tb
                        ),
                    )
                    sq = sqp.tile([P, tb, ROW_W], f32r, tag="sq")
                    nc.scalar.square(sq, xt)
                    q0 = qoff[-1]
                    sqv = sq.rearrange(
                        "p j (dh wk dw dd) -> p j dh wk dw dd", dh=2, dw=2, dd=8
                    )
                    npv = np_t[:, q0 : q0 + tb].rearrange(
                        "p j (dh wk dw) -> p j dh wk dw", dh=2, dw=2
                    )
                    for di in range(8):
                        nc.tensor.matmul(
                            npv,
                            ident,
                            sqv[:, :, :, :, :, di],
                            start=(di == 0),
                            stop=(di == 7),
                        )
                    ot = outp.tile([P, tb, 32, 8], f32, tag="ot")
                    ots.append(ot)
                    xr = xt.rearrange(
                        "p j (dh wk dw d) -> p j dh wk dw d", dh=2, dw=2, d=8
                    )
                    nc.scalar.copy(ot, xr[:, :, 1, :, 1, :])
                    qoff.append(q0 + tb)
                    tile0 += tb
                return (grp, tile0 - gtb, np_t, xts, ots, qoff)

            def stage_c(state):
                """Tournament, predicated selection, output DMA for one
                group (runs one group behind stage_a)."""
                grp, t0, np_t, xts, ots, qoff = state
                gtb = sum(grp)
                # norms layout per row: (dh, wk, dw); candidate offsets
                # A=(0,w,0)->2w, B=(0,w,1)->1+2w, C=(1,w,0)->64+2w,
                # D=(1,w,1)->65+2w
                nst = np_t.ap[1][0]  # per-j stride (=128)
                nA = _ap(np_t, 0, [[nst, gtb], [2, 32]])

                # only ONE TensorTensor input may come from PSUM, so the
                # first tournament level is a (single-input) max-reduce over
                # the dw pairs: h12[:, :, dh, wk] = max over dw.
                h12 = maskp.tile([P, gtb, 2, 32], f32, tag="h12")
                nc.vector.tensor_reduce(
                    h12.rearrange("p j dh wk -> p (j dh) wk"),
                    np_t.rearrange(
                        "p j (dh wk dw) -> p (j dh) wk dw", dh=2, dw=2
                    ),
                    axis=mybir.AxisListType.X,
                    op=mx,
                )
                M = maskp.tile([P, gtb, 32], f32, tag="M")
                nc.vector.tensor_tensor(
                    M, h12[:, :, 0], h12[:, :, 1], op=mx
                )
                wA = maskp.tile([P, gtb, 32], f32, tag="wA")
                nc.vector.tensor_tensor(wA, nA, M, op=ge)
                # combined C/B mask [P, gtb, 2, 32]: cand axis steps C -> B
                # (norm offset 64 -> 1, step -63)
                wCB = maskp.tile([P, gtb, 2, 32], f32, tag="wCB")
                nCB = _ap(np_t, 64, [[nst, gtb], [-63, 2], [2, 32]])
                Mb = _ap(M, 0, [[M.ap[1][0], gtb], [0, 2], [1, 32]])
                nc.vector.tensor_tensor(wCB, nCB, Mb, op=ge)

                tile1 = t0
                for qi, tb in enumerate(grp):
                    r0 = tile1 * P
                    xt = xts[qi]
                    ot = ots[qi]
                    q0 = qoff[qi]
                    jst = xt.ap[1][0]  # per-j stride (=1024)
                    ost = ot.ap[1][0]  # per-j stride (=256)
                    oV = _ap(ot, 0, [[ost, tb], [8, 32], [1, 8]])

                    def mask(w, off_elems, step):
                        a = w[:, q0 : q0 + tb].bitcast(i32)
                        return bass.AP(
                            tensor=a.tensor,
                            offset=a.offset + off_elems,
                            ap=[a.ap[0], [a.ap[1][0], tb], [step, 32], [0, 8]],
                        )

                    # copy_predicated is limited to 3 free dims, so C, B, A
                    # run as separate ops (overwrite order gives first-max).
                    dC = _ap(xt, 512, [[jst, tb], [16, 32], [1, 8]])
                    nc.vector.copy_predicated(oV, mask(wCB, 0, 1), dC)
                    dB = _ap(xt, 8, [[jst, tb], [16, 32], [1, 8]])
                    nc.vector.copy_predicated(oV, mask(wCB, 32, 1), dB)
                    dA = _ap(xt, 0, [[jst, tb], [16, 32], [1, 8]])
                    nc.vector.copy_predicated(oV, mask(wA, 0, 1), dA)

                    nc.sync.dma_start(
                        out=y[r0 : r0 + tb * P, :].rearrange(
                            "(p j) c -> p j c", j=tb
                        ),
                        in_=ot.rearrange("p j w d -> p j (w d)"),
                    )
                    tile1 += tb

            pending = None
            tile0 = 0
            for grp in groups:
                state = stage_a(grp, tile0)
                tile0 += sum(grp)
                if pending is not None:
                    stage_c(pending)
                pending = state
            stage_c(pending)
    nc.compile()
    return nc


_NC_CACHE = {}


def _get_nc(R):
    if R not in _NC_CACHE:
        _NC_CACHE[R] = build_nc(R)
    return _NC_CACHE[R]


def kernel(inp, kernel_size):
    inp = np.asarray(inp)
    k = int(np.asarray(kernel_size))
    assert k == 2, f"kernel hardcoded for kernel_size=2, got {k}"
    B, C, H, W, D = inp.shape
    assert (B, C, H, W, D) == (32, 32, 64, 64, 8), inp.shape
    Hk, Wk = H // k, W // k

    bs = B // N_CORES  # 4 batches per core
    R = bs * C * Hk  # 4096 rows per core
    nc = _get_nc(R)

    in_maps = []
    for c in range(N_CORES):
        shard = np.ascontiguousarray(inp[c * bs : (c + 1) * bs]).reshape(R, ROW_W)
        in_maps.append({"x": shard})

    res = run_bass_kernel_spmd(nc, in_maps, list(range(N_CORES)))
    out = np.concatenate(
        [r["y"].reshape(bs, C, Hk, Wk, D) for r in res.results], axis=0
    )
    return out
